# revision 5
# baseline (speedup 1.0000x reference)
"""Trainium2 Bass kernel for nn_CustomLSTMCell (B=65536, IN=H=256).

Data-parallel across 8 NeuronCores: the batch dim is sharded 8x8192, gate
weights replicated. On device everything is kept feature-major ("transposed",
[features, batch]) so the GEMM contraction dim (COMB=512) lands on SBUF
partitions with fully-contiguous DMA; the host wrapper transposes inputs and
outputs (numpy) around the device call.

Per core: gatesT = W @ combT via 128x128 weight-stationary float32r matmuls
(full PE rate, ~1.6e-4 rel err vs fp32 4-cycle/row mode), ScalarE applies
sigmoid/tanh(psum + bias) straight out of PSUM, VectorE does the c/h
elementwise math, 0.5-1MB batched DMAs via HWDGE.
"""

import os
import sys

import numpy as np

for _p in ("/opt/trn_rl_repo", "/root/.axon_site/_ro/trn_rl_repo"):
    if os.path.isdir(_p) and _p not in sys.path:
        sys.path.append(_p)

import concourse.bass as bass  # noqa: E402
import concourse.tile as tile  # noqa: E402
from concourse import bacc, mybir  # noqa: E402
from concourse.bass_utils import run_bass_kernel_spmd  # noqa: E402
from contextlib import ExitStack  # noqa: E402

NCORES = 8
B = 65536
IN = 256
H = 256
COMB = IN + H          # 512
NGATE = 4 * H          # 1024
BC = B // NCORES       # 8192 batch columns per core
P = 128
KT = COMB // P         # 4 contraction tiles
NT = NGATE // P        # 8 gate-feature tiles
FT = H // P            # 2 feature tiles per gate

CHUNK = 512            # matmul moving free dim (PSUM bank = [128, 512] f32)
SUPER = 2              # chunks per DMA super-chunk
CW = CHUNK * SUPER

F32 = mybir.dt.float32
F32R = mybir.dt.float32r

OUT_NAMES = ("hT_out", "cT_out", "fT_out", "iT_out", "oT_out", "gT_out")

_CACHE = {}


def _emit(tc, nc, xT, hT, cT, WT, bias, outs):
    sig = mybir.ActivationFunctionType.Sigmoid
    tanh = mybir.ActivationFunctionType.Tanh

    with ExitStack() as ctx:
        consts = ctx.enter_context(tc.tile_pool(name="consts", bufs=1))
        inp = ctx.enter_context(tc.tile_pool(name="inp", bufs=2))
        outp = ctx.enter_context(tc.tile_pool(name="outp", bufs=2))
        tmp = ctx.enter_context(tc.tile_pool(name="tmp", bufs=4))
        psum = ctx.enter_context(tc.tile_pool(name="psum", bufs=8, space="PSUM"))

        w_sb = consts.tile([P, KT, NGATE], F32R)
        nc.sync.dma_start(out=w_sb, in_=WT.rearrange("(k p) n -> p k n", p=P))
        b_sb = consts.tile([P, NT], F32)
        nc.sync.dma_start(out=b_sb, in_=bias)

        x_r = xT.rearrange("(k p) m -> p k m", p=P)    # [128, 2, BC]
        h_r = hT.rearrange("(k p) m -> p k m", p=P)    # [128, 2, BC]
        c_r = cT.rearrange("(f p) m -> p f m", p=P)    # [128, 2, BC]
        out_r = {k: v.rearrange("(f p) m -> p f m", p=P) for k, v in outs.items()}

        for s in range(BC // CW):
            ms = slice(s * CW, (s + 1) * CW)
            cx = inp.tile([P, 2, CW], F32R, tag="cx")
            nc.sync.dma_start(out=cx, in_=x_r[:, :, ms])
            ch = inp.tile([P, 2, CW], F32R, tag="ch")
            nc.sync.dma_start(out=ch, in_=h_r[:, :, ms])
            cp = inp.tile([P, FT, CW], F32, tag="cp")
            nc.sync.dma_start(out=cp, in_=c_r[:, :, ms])

            f_sb = outp.tile([P, FT, CW], F32, tag="f")
            i_sb = outp.tile([P, FT, CW], F32, tag="i")
            o_sb = outp.tile([P, FT, CW], F32, tag="o")
            g_sb = outp.tile([P, FT, CW], F32, tag="g")
            cn_sb = outp.tile([P, FT, CW], F32, tag="cn")
            hn_sb = outp.tile([P, FT, CW], F32, tag="hn")
            gate_sb = (f_sb, i_sb, o_sb, g_sb)

            for c in range(SUPER):
                cs = slice(c * CHUNK, (c + 1) * CHUNK)
                # g and i gates first: the DVE chain (f*c + i*g -> tanh -> h)
                # needs g/i/f early so the tanh(c_t) ACT op queued behind the
                # gate ACTs doesn't head-of-line-block the next chunk's gates
                # in ACT's strict FIFO.
                for n in (6, 7, 2, 3, 0, 1, 4, 5):
                    ps = psum.tile([P, CHUNK], F32, tag="ps")
                    for k in range(KT):
                        src = cx if k < 2 else ch
                        nc.tensor.matmul(
                            ps,
                            lhsT=w_sb[:, k, n * P:(n + 1) * P],
                            rhs=src[:, k % 2, cs],
                            start=(k == 0),
                            stop=(k == KT - 1),
                        )
                    nc.scalar.activation(
                        out=gate_sb[n // FT][:, n % FT, cs],
                        in_=ps,
                        func=tanh if n // FT == 3 else sig,
                        bias=b_sb[:, n:n + 1],
                        scale=1.0,
                    )
                for f in range(FT):
                    t1 = tmp.tile([P, CHUNK], F32, tag="t1")
                    nc.vector.tensor_mul(t1, f_sb[:, f, cs], cp[:, f, cs])
                    t2 = tmp.tile([P, CHUNK], F32, tag="t2")
                    nc.vector.tensor_mul(t2, i_sb[:, f, cs], g_sb[:, f, cs])
                    nc.vector.tensor_add(cn_sb[:, f, cs], t1, t2)
                    tch = tmp.tile([P, CHUNK], F32, tag="tch")
                    nc.scalar.activation(out=tch, in_=cn_sb[:, f, cs], func=tanh)
                    nc.vector.tensor_mul(hn_sb[:, f, cs], o_sb[:, f, cs], tch)

            # Stores go out via SWDGE (GpSimd) so they don't share the SP
            # HWDGE FIFO with the loads — store bursts would starve the
            # next super-chunk's input tiles and stall the PE.
            for name, t in zip(OUT_NAMES, (hn_sb, cn_sb, f_sb, i_sb, o_sb, g_sb)):
                nc.gpsimd.dma_start(out=out_r[name][:, :, ms], in_=t)


def _build():
    nc = bacc.Bacc("TRN2", target_bir_lowering=False, debug=False,
                   num_devices=NCORES)
    xT = nc.dram_tensor("xT", [IN, BC], F32R, kind="ExternalInput").ap()
    hT = nc.dram_tensor("hT", [H, BC], F32R, kind="ExternalInput").ap()
    cT = nc.dram_tensor("cT", [H, BC], F32, kind="ExternalInput").ap()
    WT = nc.dram_tensor("WT", [COMB, NGATE], F32R, kind="ExternalInput").ap()
    bias = nc.dram_tensor("bias", [P, NT], F32, kind="ExternalInput").ap()
    outs = {name: nc.dram_tensor(name, [H, BC], F32, kind="ExternalOutput").ap()
            for name in OUT_NAMES}
    with tile.TileContext(nc) as tc:
        _emit(tc, nc, xT, hT, cT, WT, bias, outs)
    nc.compile()
    return nc


LAST_RESULT = None


def kernel(x_t, h_prev, c_prev, W_f, b_f, W_i, b_i, W_o, b_o, W_g, b_g):
    global LAST_RESULT
    if "nc" not in _CACHE:
        _CACHE["nc"] = _build()
    nc = _CACHE["nc"]

    x_t = np.ascontiguousarray(x_t, dtype=np.float32)
    h_prev = np.ascontiguousarray(h_prev, dtype=np.float32)
    c_prev = np.ascontiguousarray(c_prev, dtype=np.float32)

    W = np.concatenate([W_f, W_i, W_o, W_g], axis=0).astype(np.float32)   # [1024, 512]
    WT = np.ascontiguousarray(W.T)                                        # [512, 1024]
    b = np.concatenate([b_f, b_i, b_o, b_g], axis=0).astype(np.float32)   # [1024]
    bias = np.ascontiguousarray(b.reshape(NT, P).T)                       # [128, 8]

    in_maps = []
    for i in range(NCORES):
        sl = slice(i * BC, (i + 1) * BC)
        in_maps.append({
            "xT": np.ascontiguousarray(x_t[sl].T),
            "hT": np.ascontiguousarray(h_prev[sl].T),
            "cT": np.ascontiguousarray(c_prev[sl].T),
            "WT": WT,
            "bias": bias,
        })

    trace = bool(int(os.environ.get("LSTM_TRACE", "0")))
    kw = {}
    if trace:
        try:
            import trace_hook
            trace_hook.install()
        except Exception:
            pass
        kw["trace"] = True
        tmpdir = os.environ.get("LSTM_TRACE_DIR")
        if tmpdir:
            import shutil
            shutil.rmtree(tmpdir, ignore_errors=True)
            os.makedirs(tmpdir, exist_ok=True)
            kw["tmpdir"] = tmpdir

    res = run_bass_kernel_spmd(nc, in_maps, list(range(NCORES)), **kw)
    LAST_RESULT = res

    full = {name: np.empty((B, H), dtype=np.float32) for name in OUT_NAMES}
    for i in range(NCORES):
        sl = slice(i * BC, (i + 1) * BC)
        for name in OUT_NAMES:
            full[name][sl] = res.results[i][name].T
    return tuple(full[name] for name in OUT_NAMES)


# revision 7
# speedup vs baseline: 1.1255x; 1.1255x over previous
"""Trainium2 Bass kernel for nn_CustomLSTMCell (B=65536, IN=H=256).

Data-parallel across 8 NeuronCores: the batch dim is sharded 8x8192, gate
weights replicated. On device everything is kept feature-major ("transposed",
[features, batch]) so the GEMM contraction dim (COMB=512) lands on SBUF
partitions with fully-contiguous DMA; the host wrapper transposes inputs and
outputs (numpy) around the device call.

Per core: gatesT = W @ combT via 128x128 weight-stationary float32r matmuls
(full PE rate, ~1.6e-4 rel err vs fp32 4-cycle/row mode), ScalarE applies
sigmoid/tanh(psum + bias) straight out of PSUM, VectorE does the c/h
elementwise math, 0.5-1MB batched DMAs via HWDGE.
"""

import os
import sys

import numpy as np

for _p in ("/opt/trn_rl_repo", "/root/.axon_site/_ro/trn_rl_repo"):
    if os.path.isdir(_p) and _p not in sys.path:
        sys.path.append(_p)

import concourse.bass as bass  # noqa: E402
import concourse.tile as tile  # noqa: E402
from concourse import bacc, mybir  # noqa: E402
from concourse.bass_utils import run_bass_kernel_spmd  # noqa: E402
from contextlib import ExitStack  # noqa: E402

NCORES = 8
B = 65536
IN = 256
H = 256
COMB = IN + H          # 512
NGATE = 4 * H          # 1024
BC = B // NCORES       # 8192 batch columns per core
P = 128
KT = COMB // P         # 4 contraction tiles
NT = NGATE // P        # 8 gate-feature tiles
FT = H // P            # 2 feature tiles per gate

CHUNK = 512            # matmul moving free dim (PSUM bank = [128, 512] f32)
SUPER = 2              # chunks per DMA super-chunk
CW = CHUNK * SUPER

F32 = mybir.dt.float32
F32R = mybir.dt.float32r

OUT_NAMES = ("hT_out", "cT_out", "fT_out", "iT_out", "oT_out", "gT_out")

_CACHE = {}


def _emit(tc, nc, xT, hT, cT, WT, bias, outs):
    sig = mybir.ActivationFunctionType.Sigmoid
    tanh = mybir.ActivationFunctionType.Tanh

    with ExitStack() as ctx:
        consts = ctx.enter_context(tc.tile_pool(name="consts", bufs=1))
        inp = ctx.enter_context(tc.tile_pool(name="inp", bufs=2))
        outp = ctx.enter_context(tc.tile_pool(name="outp", bufs=2))
        tmp = ctx.enter_context(tc.tile_pool(name="tmp", bufs=4))
        psum = ctx.enter_context(tc.tile_pool(name="psum", bufs=8, space="PSUM"))

        w_sb = consts.tile([P, KT, NGATE], F32R)
        nc.sync.dma_start(out=w_sb, in_=WT.rearrange("(k p) n -> p k n", p=P))
        b_sb = consts.tile([P, NT], F32)
        nc.sync.dma_start(out=b_sb, in_=bias)

        x_r = xT.rearrange("(k p) m -> p k m", p=P)    # [128, 2, BC]
        h_r = hT.rearrange("(k p) m -> p k m", p=P)    # [128, 2, BC]
        c_r = cT.rearrange("(f p) m -> p f m", p=P)    # [128, 2, BC]
        out_r = {k: v.rearrange("(f p) m -> p f m", p=P) for k, v in outs.items()}

        NSC = BC // CW
        sc_tiles = {}
        pending = None  # deferred epilogue of the previous chunk

        def epilogue(s, c):
            t = sc_tiles[s]
            cs = slice(c * CHUNK, (c + 1) * CHUNK)
            for f in range(FT):
                t1 = tmp.tile([P, CHUNK], F32, tag="t1")
                nc.vector.tensor_mul(t1, t["f"][:, f, cs], t["cp"][:, f, cs])
                t2 = tmp.tile([P, CHUNK], F32, tag="t2")
                nc.vector.tensor_mul(t2, t["i"][:, f, cs], t["g"][:, f, cs])
                nc.vector.tensor_add(t["cn"][:, f, cs], t1, t2)
                tch = tmp.tile([P, CHUNK], F32, tag="tch")
                nc.scalar.activation(out=tch, in_=t["cn"][:, f, cs], func=tanh)
                nc.vector.tensor_mul(t["hn"][:, f, cs], t["o"][:, f, cs], tch)
            if c == SUPER - 1:
                # Stores via SWDGE (GpSimd) so they don't share the SP HWDGE
                # FIFO with the loads — store bursts would starve the next
                # super-chunk's input tiles and stall the PE.
                ms = slice(s * CW, (s + 1) * CW)
                for name, key in zip(OUT_NAMES, ("hn", "cn", "f", "i", "o", "g")):
                    nc.gpsimd.dma_start(out=out_r[name][:, :, ms], in_=t[key])
                del sc_tiles[s]

        for chunk in range(NSC * SUPER):
            s, c = divmod(chunk, SUPER)
            if c == 0:
                ms = slice(s * CW, (s + 1) * CW)
                cx = inp.tile([P, 2, CW], F32R, tag="cx")
                nc.sync.dma_start(out=cx, in_=x_r[:, :, ms])
                ch = inp.tile([P, 2, CW], F32R, tag="ch")
                nc.sync.dma_start(out=ch, in_=h_r[:, :, ms])
                cp = inp.tile([P, FT, CW], F32, tag="cp")
                nc.sync.dma_start(out=cp, in_=c_r[:, :, ms])
                sc_tiles[s] = {"cx": cx, "ch": ch, "cp": cp}
                for key in ("f", "i", "o", "g", "cn", "hn"):
                    sc_tiles[s][key] = outp.tile([P, FT, CW], F32, tag=key,
                                                 name=f"{key}_{s}")
            t = sc_tiles[s]
            cs = slice(c * CHUNK, (c + 1) * CHUNK)
            gate_sb = (t["f"], t["i"], t["o"], t["g"])
            for n in range(NT):
                ps = psum.tile([P, CHUNK], F32, tag="ps")
                for k in range(KT):
                    src = t["cx"] if k < 2 else t["ch"]
                    nc.tensor.matmul(
                        ps,
                        lhsT=w_sb[:, k, n * P:(n + 1) * P],
                        rhs=src[:, k % 2, cs],
                        start=(k == 0),
                        stop=(k == KT - 1),
                    )
                nc.scalar.activation(
                    out=gate_sb[n // FT][:, n % FT, cs],
                    in_=ps,
                    func=tanh if n // FT == 3 else sig,
                    bias=b_sb[:, n:n + 1],
                    scale=1.0,
                )
            # One-chunk software pipeline: emit the previous chunk's
            # elementwise epilogue after this chunk's gates, so the tanh(c_t)
            # ACT op (which waits on the DVE chain) is never queued in front
            # of gate ACTIVATEs that PE's next PSUM group depends on.
            if pending is not None:
                epilogue(*pending)
            pending = (s, c)
        epilogue(*pending)


def _build():
    nc = bacc.Bacc("TRN2", target_bir_lowering=False, debug=False,
                   num_devices=NCORES)
    xT = nc.dram_tensor("xT", [IN, BC], F32R, kind="ExternalInput").ap()
    hT = nc.dram_tensor("hT", [H, BC], F32R, kind="ExternalInput").ap()
    cT = nc.dram_tensor("cT", [H, BC], F32, kind="ExternalInput").ap()
    WT = nc.dram_tensor("WT", [COMB, NGATE], F32R, kind="ExternalInput").ap()
    bias = nc.dram_tensor("bias", [P, NT], F32, kind="ExternalInput").ap()
    outs = {name: nc.dram_tensor(name, [H, BC], F32, kind="ExternalOutput").ap()
            for name in OUT_NAMES}
    with tile.TileContext(nc) as tc:
        _emit(tc, nc, xT, hT, cT, WT, bias, outs)
    nc.compile()
    return nc


LAST_RESULT = None


def kernel(x_t, h_prev, c_prev, W_f, b_f, W_i, b_i, W_o, b_o, W_g, b_g):
    global LAST_RESULT
    if "nc" not in _CACHE:
        _CACHE["nc"] = _build()
    nc = _CACHE["nc"]

    x_t = np.ascontiguousarray(x_t, dtype=np.float32)
    h_prev = np.ascontiguousarray(h_prev, dtype=np.float32)
    c_prev = np.ascontiguousarray(c_prev, dtype=np.float32)

    W = np.concatenate([W_f, W_i, W_o, W_g], axis=0).astype(np.float32)   # [1024, 512]
    WT = np.ascontiguousarray(W.T)                                        # [512, 1024]
    b = np.concatenate([b_f, b_i, b_o, b_g], axis=0).astype(np.float32)   # [1024]
    bias = np.ascontiguousarray(b.reshape(NT, P).T)                       # [128, 8]

    in_maps = []
    for i in range(NCORES):
        sl = slice(i * BC, (i + 1) * BC)
        in_maps.append({
            "xT": np.ascontiguousarray(x_t[sl].T),
            "hT": np.ascontiguousarray(h_prev[sl].T),
            "cT": np.ascontiguousarray(c_prev[sl].T),
            "WT": WT,
            "bias": bias,
        })

    trace = bool(int(os.environ.get("LSTM_TRACE", "0")))
    kw = {}
    if trace:
        try:
            import trace_hook
            trace_hook.install()
        except Exception:
            pass
        kw["trace"] = True
        tmpdir = os.environ.get("LSTM_TRACE_DIR")
        if tmpdir:
            import shutil
            shutil.rmtree(tmpdir, ignore_errors=True)
            os.makedirs(tmpdir, exist_ok=True)
            kw["tmpdir"] = tmpdir

    res = run_bass_kernel_spmd(nc, in_maps, list(range(NCORES)), **kw)
    LAST_RESULT = res

    full = {name: np.empty((B, H), dtype=np.float32) for name in OUT_NAMES}
    for i in range(NCORES):
        sl = slice(i * BC, (i + 1) * BC)
        for name in OUT_NAMES:
            full[name][sl] = res.results[i][name].T
    return tuple(full[name] for name in OUT_NAMES)
